# revision 44
# baseline (speedup 1.0000x reference)
"""Trainium2 Bass kernel for MatrixGraphConvolution.

out = D^-1 A (x @ W.T) + x @ B.T,  A[dst,src]=1 (set semantics),
deg counts duplicate edges, N=16384, E=524288, F=128.

Strategy (8 NeuronCores, row-sharded by dst, 2-way dst-column packing on
an integer lattice):
  * Each core owns 2048 dst rows.  The host pairs those dsts into 1024
    pairs such that no two dsts in a pair share a source (6% pairwise
    conflict rate -> greedy + relocate repair).  Each pair becomes ONE
    matmul column whose fp8e5m2 value packs two adjacency bits at
    scales {1, 4096} (both exact; 4097 never occurs by construction).
  * x is quantized to an integer lattice: xq = rint(60*x), stored as
    xq/256 in fp16 (exact).  Every PSUM partial sum is then an exact
    multiple of 1/256 with |P*256| < 2^24, so the packed accumulation
    is bit-exact and the two channels separate exactly via one
    magic-constant round (ACT) + multiply-subtract (DVE).  Lattice
    quantization error (~1/120 per element) contributes ~2e-3 max rel
    error on the output (tolerance 2e-2).
  * A-stream drops 32MB -> 16MB per core and PE columns halve vs the
    dense fp8 one-hot formulation.  W (folded with the 256/60 and 1/60
    de-scalings, one copy per channel) is applied once after
    aggregation, on top of the deg-prescaled residual x @ B.T already
    accumulated in PSUM; a final 1/deg scale covers both terms.
    Output is fp16 (upcast on host); the dst->column permutation is
    undone on the host.
"""

import sys

sys.path.insert(0, "/opt/trn_rl_repo")

import numpy as np
import ml_dtypes

import concourse.bass as bass
import concourse.tile as tile
import concourse.mybir as mybir
from concourse import bacc
from concourse.bass import ts
from concourse.bass_utils import run_bass_kernel_spmd

N, E, F = 16384, 524288, 128
NCORES = 8
SH = N // NCORES          # 2048 dst rows per core
SHB = 11                  # log2(SH)
NCH = N // 128            # 128 src chunks
GRP = 8                   # src chunks per DMA group
NGRP = NCH // GRP
XP = 8                    # x pieces (16 chunks each)
CPP = NCH // XP           # chunks per x piece

TH = 512                  # packed columns per half
CPC = 2 * TH              # packed columns per chunk (1024)
NPAIR = 1024

Q = 60.0                  # x lattice scale
XDIV = 256.0              # fp16 storage divisor for xq
S = 4096.0                # channel separation scale
MAGIC = 12582912.0        # 1.5 * 2**23: fp32 round-to-nearest-int trick
B_LO, B_HI = 0x3C, 0x6C   # fp8e5m2 bytes for 1.0 and 4096.0

FP16 = mybir.dt.float16
FP8E5 = mybir.dt.float8e5
FP32 = mybir.dt.float32

_NC = None


def _build():
    global _NC
    if _NC is not None:
        return _NC
    nc = bacc.Bacc(None, target_bir_lowering=False)
    # at is pre-tiled: at[g][p][j*CPC + col] = Apk[src=(g*GRP+j)*128 + p, col]
    at = nc.dram_tensor("at", [NGRP, 128, GRP * CPC], FP8E5, kind="ExternalInput")
    # xp is pre-tiled lattice x: xp[i][p][q*F + f] = rint(Q*x[(i*CPP+q)*128+p, f])/XDIV
    xp = nc.dram_tensor("xp", [XP, 128, CPP * F], FP16, kind="ExternalInput")
    # starter pack bytes: [x chunks 0-1 as fp16 | A chunks 0-1 as fp8e5m2]
    st0 = nc.dram_tensor(
        "st0", [128, 2 * F * 2 + 2 * CPC], mybir.dt.uint8, kind="ExternalInput"
    )
    xtc = nc.dram_tensor("xtc", [F, SH], FP16, kind="ExternalInput")
    # cst packs [bt | wtl | wth] to issue one early DMA instead of three
    cst = nc.dram_tensor("cst", [F, 3 * F], FP16, kind="ExternalInput")
    idr = nc.dram_tensor("idr", [1, SH], FP32, kind="ExternalInput")
    outT = nc.dram_tensor("outT", [F, SH], FP16, kind="ExternalOutput")

    with tile.TileContext(nc) as tc:
        with (
            tc.tile_pool(name="const", bufs=1) as constp,
            tc.tile_pool(name="apool", bufs=8) as apool,
            tc.tile_pool(name="outp", bufs=1) as outp,
            tc.tile_pool(name="psA", bufs=1, space=bass.MemorySpace.PSUM) as psA,
        ):
            # one consolidated const DMA on the Scalar ring; xtc rides the
            # Sync ring AHEAD of the A stream; idr issues from the idle
            # Vector ring -- parallel issue, ~1us SWDGE overhead each.
            cst_sb = constp.tile([F, 3 * F], FP16, tag="cst")
            nc.scalar.dma_start(cst_sb[:], cst[:])
            bt_sb = cst_sb[:, 0:F]
            wtl_sb = cst_sb[:, F : 2 * F]
            wth_sb = cst_sb[:, 2 * F : 3 * F]
            xtc_sb = constp.tile([F, SH], FP16, tag="xtc")
            idr_sb = constp.tile([1, SH], FP32, tag="idr")
            nc.gpsimd.dma_start(idr_sb[:], idr[:])
            # starter pack: x chunks 0-1 + A chunks 0-1 fused in ONE uint8
            # DMA (one SWDGE issue, guaranteed co-arrival) via bitcast views
            st0_sb = constp.tile([128, 2 * F * 2 + 2 * CPC], mybir.dt.uint8, tag="st0")
            xp0a_sb = st0_sb[:, 0 : 2 * F * 2].bitcast(FP16)
            a0a = st0_sb[:, 2 * F * 2 :].bitcast(FP8E5)
            xp0b_sb = constp.tile([128, (CPP - 2) * F], FP16, tag="xp0b")
            xp_sb = [None] + [
                constp.tile([128, CPP * F], FP16, tag=f"xp{i}", name=f"xp{i}")
                for i in range(1, XP)
            ]

            def xp_slice(c):
                i, q = divmod(c, CPP)
                if i == 0:
                    if q < 2:
                        return xp0a_sb[:, ts(q, F)]
                    return xp0b_sb[:, ts(q - 2, F)]
                return xp_sb[i][:, ts(q, F)]

            psb = [
                psA.tile([128, 512], FP32, tag=f"agg{b}", name=f"psb{b}")
                for b in range(4)
            ]                                              # res + W @ Y
            psy = [
                psA.tile([128, TH], FP32, tag=f"y{h}", name=f"psy{h}")
                for h in range(2)
            ]

            # invdeg partition-broadcast early (GpSimd is otherwise idle)
            idb_sb = constp.tile([128, SH], FP32, tag="idb")
            nc.gpsimd.partition_broadcast(idb_sb[:], idr_sb[:])

            # main loop: Ypk[c, col] += sum_src xq[src, c] * Apk[src, col]
            # x pieces ride the Sync ring interleaved with the A groups so
            # the A stream is never starved by a front-loaded 4MB x transfer.
            nc.sync.dma_start(st0_sb[:], st0[:])
            nc.sync.dma_start(xp0b_sb[:], xp[0][:, 2 * F : CPP * F])
            for g in range(NGRP):
                if g == 0:
                    a_t = apool.tile([128, (GRP - 2) * CPC], FP8E5, tag="a0b")
                    nc.sync.dma_start(a_t[:], at[0][:, 2 * CPC : GRP * CPC])
                else:
                    a_t = apool.tile([128, GRP * CPC], FP8E5, tag="a")
                    nc.sync.dma_start(a_t[:], at[g])
                if g % 2 == 1 and (g + 1) // 2 < XP:
                    i = (g + 1) // 2
                    nc.sync.dma_start(xp_sb[i][:], xp[i])
                if g == 8:
                    # xtc rides mid-stream: ready long before the residual
                    # matmuls, without delaying the kernel start
                    nc.sync.dma_start(xtc_sb[:], xtc[:])
                if g == 10:
                    # residual off the tail critical path: PE absorbs these
                    # four matmuls during a mid-stream DMA-lead window
                    for b in range(4):
                        nc.tensor.matmul(
                            psb[b][:],
                            bt_sb,
                            xtc_sb[:, ts(b, 512)],
                            start=True,
                            stop=False,
                        )
                if g < NGRP - 1:
                    hj = [(h, j) for j in range(GRP) for h in range(2)]
                else:
                    # last group: finish half 1 first so its unpack overlaps
                    # half 0's final matmuls
                    hj = [(h, j) for h in (1, 0) for j in range(GRP)]
                for h, j in hj:
                    c = g * GRP + j
                    if g == 0 and j < 2:
                        mov = a0a[:, bass.ds(j * CPC + h * TH, TH)]
                    elif g == 0:
                        mov = a_t[:, bass.ds((j - 2) * CPC + h * TH, TH)]
                    else:
                        mov = a_t[:, bass.ds(j * CPC + h * TH, TH)]
                    nc.tensor.matmul(
                        psy[h][:],
                        xp_slice(c),
                        mov,
                        start=(c == 0),
                        stop=(c == NCH - 1),
                    )

            # tail: separate 2 channels per half (half 1 finishes first),
            # apply W on top of the residual, 1/deg scale, DMA out.
            # Engine split: ACT rounds/casts, DVE extracts lo, GpSimd scales.
            out_sb = outp.tile([128, SH], FP16, tag="out")
            Copy = mybir.ActivationFunctionType.Copy
            y16s = [
                constp.tile([128, 2 * TH], FP16, tag=f"y16_{h}", name=f"y16_{h}")
                for h in range(2)
            ]
            # half 1 unpacks on ACT (psy1 stops ~8 matmuls early), half 0 on
            # DVE -- the two chains run on different engines in parallel so
            # half 0's unpack starts the moment psy0 stops.
            for h in (1, 0):
                P = psy[h][:]
                u = constp.tile([128, TH], FP32, tag=f"u{h}", name=f"u{h}")
                nh = constp.tile([128, TH], FP32, tag=f"nh{h}", name=f"nh{h}")
                y16 = y16s[h]
                if h == 1:
                    # u = P*(XDIV/S) + M; nh = (u-M)*(-S/XDIV)  (ACT, exact)
                    nc.scalar.activation(u[:], P, Copy, bias=MAGIC, scale=XDIV / S)
                    nc.scalar.activation(
                        nh[:], u[:], Copy, bias=MAGIC * (S / XDIV), scale=-S / XDIV
                    )
                else:
                    nc.vector.tensor_scalar(
                        u[:], P, XDIV / S, MAGIC,
                        mybir.AluOpType.mult, mybir.AluOpType.add,
                    )
                    nc.vector.tensor_scalar(
                        nh[:], u[:], -MAGIC, -S / XDIV,
                        mybir.AluOpType.add, mybir.AluOpType.mult,
                    )
                # y16_hi = u - M  (integer <= ~2048, exact in fp16; ACT)
                nc.scalar.activation(
                    y16[:, bass.ds(TH, TH)], u[:], Copy, bias=-MAGIC, scale=1.0
                )
                # y16_lo = P + nh  (= Lo/XDIV; DVE)
                nc.vector.tensor_add(y16[:, 0:TH], P, nh[:])
            for h in (1, 0):
                for k in range(2):
                    nc.tensor.matmul(
                        psb[2 * h + k][:],
                        wtl_sb if k == 0 else wth_sb,
                        y16s[h][:, ts(k, 512)],
                        start=False,
                        stop=True,
                    )
                    nc.vector.tensor_mul(
                        out_sb[:, bass.ds(1024 * h + 512 * k, 512)],
                        psb[2 * h + k][:],
                        idb_sb[:, bass.ds(1024 * h + 512 * k, 512)],
                    )
                    nc.scalar.dma_start(
                        outT[:, bass.ds(1024 * h + 512 * k, 512)],
                        out_sb[:, bass.ds(1024 * h + 512 * k, 512)],
                    )

    nc.compile()
    _NC = nc
    return nc


def _make_pairs(srcsets):
    """Pair 2048 dsts into 1024 pairs with no shared src within a pair."""
    nd = len(srcsets)
    order = sorted(range(nd), key=lambda d: -len(srcsets[d]))
    pairs = []            # complete pairs
    opens = []            # singletons awaiting a partner
    for d in order:
        s = srcsets[d]
        placed = False
        for i, (d0, s0) in enumerate(opens):
            if not (s0 & s):
                pairs.append((d0, d))
                opens.pop(i)
                placed = True
                break
        if not placed:
            opens.append((d, s))
    # repair: leftover singletons conflict with every open one; break an
    # existing pair whose members can host them
    while len(opens) >= 2:
        d0, s0 = opens.pop()
        fixed = False
        for pi, (a, b) in enumerate(pairs):
            if srcsets[a] & s0:
                continue
            # try to re-home b with another open singleton
            for i, (d1, s1) in enumerate(opens):
                if not (srcsets[b] & s1):
                    pairs[pi] = (a, d0)
                    pairs.append((b, d1))
                    opens.pop(i)
                    fixed = True
                    break
            if fixed:
                break
        if not fixed:
            raise RuntimeError("pair repair failed")
    assert not opens and len(pairs) == NPAIR
    return pairs


def _prep_inputs(x, edge_index, W, B):
    src = np.asarray(edge_index[0]).astype(np.int64)
    dst = np.asarray(edge_index[1]).astype(np.int64)
    x = np.asarray(x, dtype=np.float32)
    W = np.asarray(W, dtype=np.float32)
    B = np.asarray(B, dtype=np.float32)

    deg = np.bincount(dst, minlength=N).astype(np.float32)
    dtil = np.where(deg == 0, np.float32(1.0), deg)
    invdeg = (np.float32(1.0) / dtil).astype(np.float32)

    # dedup edges (set semantics for A)
    ukey = np.unique(dst * N + src)
    us = (ukey % N).astype(np.int64)
    ud = (ukey // N).astype(np.int64)

    # per-dst packed column and channel
    pcol = np.empty(N, dtype=np.int64)       # packed psum column (0..CPC)
    colpos = np.empty(N, dtype=np.int64)     # position in out/y16 layout
    chval = np.empty(N, dtype=np.uint8)
    osort = np.argsort(ud, kind="stable")
    us_s, ud_s = us[osort], ud[osort]
    for k in range(NCORES):
        base = k * SH
        elo = np.searchsorted(ud_s, base)
        ehi = np.searchsorted(ud_s, base + SH)
        es = us_s[elo:ehi]
        edl = (ud_s[elo:ehi] - base).astype(np.int64)
        bounds = np.searchsorted(edl, np.arange(SH + 1))
        srcsets = [
            frozenset(es[bounds[i] : bounds[i + 1]].tolist()) for i in range(SH)
        ]
        pairs = _make_pairs(srcsets)
        for c, (dlo, dhi) in enumerate(pairs):
            h, j = divmod(c, TH)
            for ci, d in enumerate((dlo, dhi)):
                gd = base + d
                pcol[gd] = c
                chval[gd] = (B_LO, B_HI)[ci]
                colpos[gd] = 1024 * h + TH * ci + j

    # packed A, pre-tiled: at_all[core, g, p, j*CPC + pcol]
    at_all = np.zeros((NCORES, NGRP, 128, GRP * CPC), dtype=np.uint8)
    g = us >> 10
    p = us & 127
    j = (us >> 7) & (GRP - 1)
    at_all[ud >> SHB, g, p, (j * CPC) + pcol[ud]] = chval[ud]
    at_all = at_all.view(ml_dtypes.float8_e5m2)

    # lattice x, pre-tiled for stationary chunks: [XP, 128, CPP*F]
    xq = (np.rint(Q * x) / np.float32(XDIV)).astype(np.float16)
    xp_np = np.ascontiguousarray(
        xq.reshape(XP, CPP, 128, F).transpose(0, 2, 1, 3)
    ).reshape(XP, 128, CPP * F)
    xtil = (dtil[:, None] * x).astype(np.float16)
    cst_np = np.ascontiguousarray(
        np.concatenate(
            [B.T, (W * (XDIV / Q)).T, (W * (1.0 / Q)).T], axis=1
        )
    ).astype(np.float16)

    xp_bytes = xp_np.view(np.uint8)
    at_bytes = at_all.view(np.uint8)
    in_maps = []
    perms = []
    for k in range(NCORES):
        sl = slice(k * SH, (k + 1) * SH)
        cp = colpos[sl]
        xtc_k = np.zeros((SH, F), dtype=np.float16)
        xtc_k[cp] = xtil[sl]
        idr_k = np.zeros((1, SH), dtype=np.float32)
        idr_k[0, cp] = invdeg[sl]
        st0_k = np.ascontiguousarray(
            np.concatenate(
                [xp_bytes[0][:, 0 : 2 * F * 2], at_bytes[k][0][:, 0 : 2 * CPC]],
                axis=1,
            )
        )
        in_maps.append(
            {
                "at": at_all[k],
                "xp": xp_np,
                "st0": st0_k,
                "xtc": np.ascontiguousarray(xtc_k.T),
                "cst": cst_np,
                "idr": idr_k,
            }
        )
        perms.append(cp)
    return in_maps, perms


def _assemble(results, perms):
    out = np.empty((N, F), dtype=np.float32)
    for k in range(NCORES):
        outT = np.asarray(results[k]["outT"]).astype(np.float32)
        out[k * SH : (k + 1) * SH, :] = outT.T[perms[k]]
    return out


def kernel(x, edge_index, W, B):
    nc = _build()
    in_maps, perms = _prep_inputs(x, edge_index, W, B)
    res = run_bass_kernel_spmd(nc, in_maps, core_ids=list(range(NCORES)))
    return _assemble(res.results, perms)
